# revision 2
# baseline (speedup 1.0000x reference)
"""Trainium2 Bass kernel for nn_DiffusionModel (auction-matched flow targets), v2.

Self-contained: accepts FULL inputs (cloud [16,2048,3], noise [16,2048,3],
t [16]), shards batch over 8 NeuronCores (2 samples per core), runs the full
5-iteration bijective auction per sample on device, returns [2,16,2048,3].

v2 design (vs baseline):
  - PE recomputes Vp = -(dist + price) every iteration as a K=5 fp32 matmul
    (price folded into the yn row of the rhs; -|n|^2 via a rank-1 ones term)
    directly into PSUM; DVE does only max/max_index from PSUM.
  - price kept as a [1,N] row (partition 32); scatter-max via per-partition
    dedup + local_scatter of f32 bid bit-planes + partition_all_reduce lex
    max; plane combine + price-row update run on gpsimd.
  - x0_aligned gather via indirect_dma_start from a DRAM copy of cloud.
  - the two samples are software-pipelined so DVE never idles.
"""
import numpy as np

P = 128
N = 2048
NG = 16          # row groups per sample (NG * P = N rows)
D = 3
SPC = 2          # samples per core
EPS = 1e-3
NCORES = 8
REPEAT = 1       # benchmark knob: repeat the whole per-core pipeline
NITER = 5


def _build_program():
    import concourse.bass as bass
    import concourse.tile as tile
    from concourse import bacc, mybir, bass_isa
    import os

    fp32 = mybir.dt.float32
    u16 = mybir.dt.uint16
    i16 = mybir.dt.int16
    i32 = mybir.dt.int32
    u32 = mybir.dt.uint32
    OP = mybir.AluOpType
    AX = mybir.AxisListType
    AF = mybir.ActivationFunctionType

    DBG = bool(int(os.environ.get("BASSDBG", "0")))

    nc = bacc.Bacc("TRN2", target_bir_lowering=False, debug=False,
                   enable_asserts=False)

    # ---- DRAM I/O ----
    # noiseTn: -noise^T  [3, N] (negated on host for the lhsT rows)
    noiseTn_d = nc.dram_tensor("noiseTn", [SPC, 3, N], fp32, kind="ExternalInput")
    cloudT_d = nc.dram_tensor("cloudT", [SPC, 3, N], fp32, kind="ExternalInput")
    cloudR_d = nc.dram_tensor("cloudR", [SPC, P, NG * D], fp32, kind="ExternalInput")
    noiseR_d = nc.dram_tensor("noiseR", [SPC, P, NG * D], fp32, kind="ExternalInput")
    cloudJ_d = [nc.dram_tensor(f"cloudJ{s}", [N, D], fp32, kind="ExternalInput")
                for s in range(SPC)]
    tv_d = nc.dram_tensor("tv", [SPC, 1], fp32, kind="ExternalInput")
    ltc_d = nc.dram_tensor("ltc", [P, NG * NG], u16, kind="ExternalInput")
    # consts row 0: ones, row 1: -1
    consts_d = nc.dram_tensor("consts", [2, N], fp32, kind="ExternalInput")
    out_d = nc.dram_tensor("out", [SPC, 2, P, NG * D], fp32, kind="ExternalOutput")
    if DBG:
        dbg_top_d = nc.dram_tensor("dbg_top", [SPC, NITER, P, NG * 8], fp32,
                                   kind="ExternalOutput")
        dbg_idx_d = nc.dram_tensor("dbg_idx", [SPC, NITER, P, NG * 8], u16,
                                   kind="ExternalOutput")
        dbg_pr_d = nc.dram_tensor("dbg_pr", [SPC, NITER, 1, N], fp32,
                                  kind="ExternalOutput")

    with tile.TileContext(nc) as tc:
        with (
            tc.tile_pool(name="sb", bufs=1) as sp,
            tc.tile_pool(name="ps", bufs=2, space="PSUM") as psA,
        ):
            # ---- shared constants ----
            LTC = sp.tile([P, NG * NG], u16, tag="ltc")
            nc.sync.dma_start(LTC[:], ltc_d.ap())
            ones128 = sp.tile([P, 1], fp32, tag="ones128")
            nc.vector.memset(ones128[:], 1.0)
            ones3 = sp.tile([3, 1], fp32, tag="ones3")
            nc.vector.memset(ones3[:], 1.0)
            MINUS1 = sp.tile([P, NG], fp32, tag="minus1")
            nc.vector.memset(MINUS1[:], -1.0)
            ZROW = sp.tile([1, N], fp32, tag="zrow")
            nc.vector.memset(ZROW[:], 0.0)

            # ---- per-sample persistent tiles ----
            lhsT = [sp.tile([37, N], fp32, tag=f"lhsT{s}", name=f"lhsT{s}") for s in range(SPC)]
            rhs = [sp.tile([37, N], fp32, tag=f"rhs{s}", name=f"rhs{s}") for s in range(SPC)]
            nR = [sp.tile([P, NG * D], fp32, tag=f"nR{s}", name=f"nR{s}") for s in range(SPC)]
            stdb = [sp.tile([P, 1], fp32, tag=f"stdb{s}", name=f"stdb{s}") for s in range(SPC)]
            TOP8 = [sp.tile([P, NG * 8], fp32, tag=f"top8{s}", name=f"top8{s}") for s in range(SPC)]
            IDX8 = [sp.tile([P, NG * 8], u16, tag=f"idx8{s}", name=f"idx8{s}") for s in range(SPC)]
            # scatter-side state
            MHI = [sp.tile([P, N], u16, tag=f"mhi{s}", name=f"mhi{s}") for s in range(SPC)]
            MLO = [sp.tile([P, N], u16, tag=f"mlo{s}", name=f"mlo{s}") for s in range(SPC)]
            CHI = [sp.tile([P, N], u16, tag=f"chi{s}", name=f"chi{s}") for s in range(SPC)]
            CLO = [sp.tile([P, N], u16, tag=f"clo{s}", name=f"clo{s}") for s in range(SPC)]
            SLO = [sp.tile([P, N], u16, tag=f"slo{s}", name=f"slo{s}") for s in range(SPC)]
            # rows live at partition 32 of [33, N] strips
            PB32 = [sp.tile([1, N], i32, tag=f"pb32{s}", name=f"pb32{s}") for s in range(SPC)]
            SPR0 = [sp.tile([1, N], fp32, tag=f"spr{s}", name=f"spr{s}") for s in range(SPC)]
            MASK0 = [sp.tile([1, N], u16, tag=f"mask{s}", name=f"mask{s}") for s in range(SPC)]

            def prep(s):
                """DMA inputs, compute std, yn row, xn row, build strips."""
                # lhsT rows: 32 = -1 (price), 33 = -1 (yn), 34-36 = -noise^T
                nc.sync.dma_start(lhsT[s][34:37, :], noiseTn_d.ap()[s])
                nc.sync.dma_start(lhsT[s][32:33, :], consts_d.ap()[1:2, :])
                nc.sync.dma_start(lhsT[s][33:34, :], consts_d.ap()[1:2, :])
                CT0 = sp.tile([3, N], fp32, tag=f"ct0_{s}")
                nc.sync.dma_start(CT0[:], cloudT_d.ap()[s])
                CSQ = sp.tile([3, N], fp32, tag=f"ct0_{s}", name=f"csq{s}")
                cR = sp.tile([P, NG * D], fp32, tag=f"cR{s}")
                nc.sync.dma_start(cR[:], cloudR_d.ap()[s])
                nc.sync.dma_start(nR[s][:], noiseR_d.ap()[s])

                pp = psA.tile([P, N], fp32, tag="vp")
                # ---- std (two-pass, ddof=1) ----
                red = sp.tile([P, 1], fp32, tag=f"red{s}")
                nc.vector.tensor_reduce(red[:], cR[:], axis=AX.X, op=OP.add)
                pm = pp[0:1, 0:1]
                nc.tensor.matmul(pm, red[:], ones128[:])
                negmean = sp.tile([1, 1], fp32, tag=f"negmean{s}")
                nc.scalar.activation(negmean[:], pm, AF.Identity,
                                     bias=0.0, scale=-1.0 / (N * D))
                negmeanb = sp.tile([P, 1], fp32, tag=f"negmeanb{s}")
                nc.gpsimd.partition_broadcast(negmeanb[:], negmean[:], channels=P)
                sqdev = sp.tile([P, NG * D], fp32, tag=f"sqdev{s}")
                nc.scalar.activation(sqdev[:], cR[:], AF.Square,
                                     bias=negmeanb[:], scale=1.0)
                red2 = sp.tile([P, 1], fp32, tag=f"red2{s}")
                nc.vector.tensor_reduce(red2[:], sqdev[:], axis=AX.X, op=OP.add)
                pv = pp[0:1, 2:3]
                nc.tensor.matmul(pv, red2[:], ones128[:])
                var1 = sp.tile([1, 1], fp32, tag=f"var1{s}")
                nc.scalar.activation(var1[:], pv, AF.Identity,
                                     bias=0.0, scale=1.0 / (N * D - 1))
                std1 = sp.tile([1, 1], fp32, tag=f"std1{s}")
                nc.scalar.activation(std1[:], var1[:], AF.Sqrt,
                                     bias=0.0, scale=1.0)
                invvar = sp.tile([1, 1], fp32, tag=f"invvar{s}")
                nc.vector.reciprocal(invvar[:], var1[:])
                invstd = sp.tile([1, 1], fp32, tag=f"invstd{s}")
                nc.vector.reciprocal(invstd[:], std1[:])
                nc.gpsimd.partition_broadcast(stdb[s][:], invstd[:], channels=P)

                # ---- rhs coord rows (34-36) = cloudT * (-2 * invstd) ----
                nc.vector.tensor_scalar(CT0[:], CT0[:], stdb[s][0:3, :], -2.0,
                                        op0=OP.mult, op1=OP.mult)
                nc.sync.dma_start(rhs[s][34:37, :], CT0[:])
                # NOTE: CT0 now holds scaled coords; csq comes fresh (same buffer)
                nc.sync.dma_start(CSQ[:], cloudT_d.ap()[s])
                nc.scalar.activation(CSQ[:], CSQ[:], AF.Square, bias=0.0,
                                     scale=1.0)
                # yn row = sum(cloudT^2) / var  -> ynrow (p0) and YPR init (p32)
                for t in range(4):
                    pyn = pp[0:1, 512 * t:512 * (t + 1)]
                    nc.tensor.matmul(pyn, ones3[:],
                                     CSQ[:, 512 * t:512 * (t + 1)])
                    nc.scalar.activation(rhs[s][0:1, 512 * t:512 * (t + 1)],
                                         pyn, AF.Identity, bias=0.0,
                                         scale=invvar[:])
                # rhs row 33 = yn (fixed); row 32 = price (starts at 0)
                nc.sync.dma_start(rhs[s][33:34, :], rhs[s][0:1, :])
                nc.vector.memset(SPR0[s][:], 0.0)
                nc.scalar.activation(rhs[s][32:33, :], SPR0[s][:],
                                     AF.Identity, bias=0.0, scale=1.0)

            def scan(s, it, mid=None):
                """PE Vp matmuls + DVE max/max_index per group."""
                for g in range(NG):
                    if g == 8 and mid is not None:
                        mid()
                    vp = psA.tile([P, N], fp32, tag="vp")
                    for t in range(4):
                        nc.tensor.matmul(
                            vp[:, 512 * t:512 * (t + 1)],
                            lhsT[s][32:37, P * g:P * (g + 1)],
                            rhs[s][32:37, 512 * t:512 * (t + 1)])
                    nc.vector.max(TOP8[s][:, 8 * g:8 * (g + 1)], vp[:])
                    nc.vector.max_index(IDX8[s][:, 8 * g:8 * (g + 1)],
                                        TOP8[s][:, 8 * g:8 * (g + 1)], vp[:])
                if DBG:
                    nc.sync.dma_start(dbg_top_d.ap()[s, it], TOP8[s][:])
                    nc.sync.dma_start(dbg_idx_d.ap()[s, it], IDX8[s][:])
                    nc.sync.dma_start(dbg_pr_d.ap()[s, it], rhs[s][32:33, :])

            def bid_scatter(s):
                """DVE dedup smalls + gpsimd scatter/reduce chain."""
                t8v = TOP8[s][:].rearrange("p (g k) -> p g k", k=8)
                m1 = t8v[:, :, 0]
                m2 = t8v[:, :, 1]
                jsel = IDX8[s][:].rearrange("p (g k) -> p g k", k=8)[:, :, 0]
                JF = sp.tile([P, NG], fp32, tag=f"jf{s}")
                nc.vector.tensor_copy(JF[:], jsel)
                BIDF = sp.tile([P, NG], fp32, tag=f"bidf{s}")
                # bid = (m1 + eps) - m2
                nc.vector.scalar_tensor_tensor(BIDF[:], m1, float(EPS), m2,
                                               op0=OP.add, op1=OP.subtract)

                # ---- dedup within partition (16 bids each) ----
                ja = JF[:].unsqueeze(2).broadcast_to([P, NG, NG])
                jb = JF[:].unsqueeze(1).broadcast_to([P, NG, NG])
                ba = BIDF[:].unsqueeze(2).broadcast_to([P, NG, NG])
                bb = BIDF[:].unsqueeze(1).broadcast_to([P, NG, NG])
                dA = sp.tile([P, NG * NG], u16, tag=f"dA{s}")
                dB = sp.tile([P, NG * NG], u16, tag=f"dB{s}")
                dC = sp.tile([P, NG * NG], u16, tag=f"dC{s}")
                dAv = dA[:].rearrange("p (a b) -> p a b", b=NG)
                dBv = dB[:].rearrange("p (a b) -> p a b", b=NG)
                dCv = dC[:].rearrange("p (a b) -> p a b", b=NG)
                nc.vector.tensor_tensor(dAv, jb, ja, op=OP.is_equal)
                nc.vector.tensor_tensor(dBv, bb, ba, op=OP.is_gt)
                nc.vector.tensor_tensor(dCv, bb, ba, op=OP.is_equal)
                ltcv = LTC[:].rearrange("p (a b) -> p a b", b=NG)
                nc.vector.tensor_tensor(dCv, dCv, ltcv, op=OP.mult)
                nc.vector.tensor_tensor(dBv, dBv, dCv, op=OP.max)
                nc.vector.tensor_tensor(dAv, dAv, dBv, op=OP.mult)
                KILL = sp.tile([P, NG], u16, tag=f"kill{s}")
                nc.vector.tensor_reduce(KILL[:], dAv, axis=AX.X, op=OP.max)
                JEFF = sp.tile([P, NG], fp32, tag=f"jeff{s}")
                nc.vector.select(JEFF[:], KILL[:], MINUS1[:], JF[:])

                # ---- halves + int16 indices ----
                GEH = sp.tile([P, NG], u16, tag=f"geh{s}")
                nc.vector.tensor_scalar(GEH[:], JEFF[:], 1024.0, None,
                                        op0=OP.is_ge)
                JAf = sp.tile([P, NG], fp32, tag=f"jaf{s}")
                JBm = sp.tile([P, NG], fp32, tag=f"jbm{s}")
                JBf = sp.tile([P, NG], fp32, tag=f"jbf{s}")
                nc.vector.select(JAf[:], GEH[:], MINUS1[:], JEFF[:])
                nc.vector.tensor_scalar(JBm[:], JEFF[:], -1024.0, None,
                                        op0=OP.add)
                nc.vector.select(JBf[:], GEH[:], JBm[:], MINUS1[:])
                JA16 = sp.tile([P, NG], i16, tag=f"ja16{s}")
                JB16 = sp.tile([P, NG], i16, tag=f"jb16{s}")
                nc.vector.tensor_copy(JA16[:], JAf[:])
                nc.vector.tensor_copy(JB16[:], JBf[:])

                # ---- bid bit-planes ----
                bbits = BIDF[:].bitcast(u16).rearrange(
                    "p (k two) -> p k two", two=2)
                BLO = sp.tile([P, NG], u16, tag=f"blo{s}")
                BHI = sp.tile([P, NG], u16, tag=f"bhi{s}")
                nc.vector.tensor_copy(BLO[:], bbits[:, :, 0])
                nc.vector.tensor_copy(BHI[:], bbits[:, :, 1])

                # ---- gpsimd: dense scatter + partition max (lexicographic) ----
                for half, idxs in ((0, JA16), (1, JB16)):
                    nc.gpsimd.local_scatter(
                        MHI[s][:, 1024 * half:1024 * (half + 1)], BHI[:],
                        idxs[:], channels=P, num_elems=1024, num_idxs=NG)
                    nc.gpsimd.local_scatter(
                        MLO[s][:, 1024 * half:1024 * (half + 1)], BLO[:],
                        idxs[:], channels=P, num_elems=1024, num_idxs=NG)
                nc.gpsimd.partition_all_reduce(CHI[s][:], MHI[s][:], channels=P,
                                               reduce_op=bass_isa.ReduceOp.max)

            def eqd_slo(s):
                """DVE lex-combine stage (after first all-reduce)."""
                nc.vector.tensor_tensor(SLO[s][:], MHI[s][:], CHI[s][:],
                                        op=OP.is_equal)
                nc.vector.tensor_tensor(SLO[s][:], MLO[s][:], SLO[s][:],
                                        op=OP.mult)
                nc.gpsimd.partition_all_reduce(CLO[s][:], SLO[s][:], channels=P,
                                               reduce_op=bass_isa.ReduceOp.max)
                # combine planes into f32 bid row + SUM = ynrow + newbid (gpsimd)
                pnew16 = PB32[s][:].bitcast(u16).rearrange(
                    "p (n two) -> p n two", two=2)
                nc.gpsimd.tensor_copy(pnew16[:, :, 0], CLO[s][0:1, :])
                nc.gpsimd.tensor_copy(pnew16[:, :, 1], CHI[s][0:1, :])

            def rows2(s):
                """DVE mask + predicated price-row replace, ACT copy to p32."""
                nc.vector.tensor_scalar(MASK0[s][:], PB32[s][:].bitcast(fp32),
                                        0.0, None, op0=OP.is_gt)
                nc.vector.copy_predicated(SPR0[s][:], MASK0[s][:],
                                          PB32[s][:].bitcast(fp32))
                nc.scalar.activation(rhs[s][32:33, :], SPR0[s][:],
                                     AF.Identity, bias=0.0, scale=1.0)

            def output(s):
                """Gather x0[jstar] via indirect DMA, combine, DMA out."""
                jsel = IDX8[s][:].rearrange("p (g k) -> p g k", k=8)[:, :, 0]
                JIDX = sp.tile([P, NG], u32, tag=f"jidx{s}")
                nc.vector.tensor_copy(JIDX[:], jsel)
                X0G = sp.tile([P, NG, D], fp32, tag=f"x0g{s}")
                for g in range(NG):
                    nc.gpsimd.indirect_dma_start(
                        out=X0G[:, g, :], out_offset=None,
                        in_=cloudJ_d[s].ap(),
                        in_offset=bass.IndirectOffsetOnAxis(
                            ap=JIDX[:, g:g + 1], axis=0),
                    )
                x0a = sp.tile([P, NG * D], fp32, tag=f"x0a{s}")
                nc.vector.tensor_scalar(
                    x0a[:], X0G[:].rearrange("p g d -> p (g d)"), stdb[s][:],
                    None, op0=OP.mult)
                tb1 = sp.tile([1, 1], fp32, tag=f"tb1{s}")
                nc.sync.dma_start(tb1[:], tv_d.ap()[s].unsqueeze(0))
                TB = sp.tile([P, 1], fp32, tag=f"tbb{s}")
                nc.gpsimd.partition_broadcast(TB[:], tb1[:], channels=P)
                OMT = sp.tile([P, 1], fp32, tag=f"omt{s}")
                nc.vector.tensor_scalar(OMT[:], TB[:], -1.0, 1.0,
                                        op0=OP.mult, op1=OP.add)
                NTt = sp.tile([P, NG * D], fp32, tag=f"ntt{s}")
                XT = sp.tile([P, NG * D], fp32, tag=f"xt{s}")
                VV = sp.tile([P, NG * D], fp32, tag=f"vv{s}")
                nc.vector.tensor_scalar(NTt[:], nR[s][:], TB[:], None,
                                        op0=OP.mult)
                nc.vector.scalar_tensor_tensor(XT[:], x0a[:], OMT[:], NTt[:],
                                               op0=OP.mult, op1=OP.add)
                nc.vector.tensor_tensor(VV[:], nR[s][:], x0a[:],
                                        op=OP.subtract)
                nc.sync.dma_start(out_d.ap()[s, 0], XT[:])
                nc.sync.dma_start(out_d.ap()[s, 1], VV[:])

            # ================= program =================
            for rep in range(REPEAT):
                for s in range(SPC):
                    prep(s)
                # pending[o] = True when sample o has a scatter chain in flight
                pending = [False] * SPC
                for it in range(NITER):
                    for s in range(SPC):
                        o = 1 - s
                        scan(s, it, mid=(lambda: eqd_slo(o)) if pending[o]
                             else None)
                        if pending[o]:
                            rows2(o)
                            pending[o] = False
                        if it < NITER - 1:
                            bid_scatter(s)
                            pending[s] = True
                        else:
                            output(s)

    nc.compile()
    return nc


_NC_CACHE = None


def _get_nc():
    global _NC_CACHE
    if _NC_CACHE is None:
        _NC_CACHE = _build_program()
    return _NC_CACHE


def _host_prep(cloud, noise, t):
    """Build per-core input maps."""
    ltc = np.zeros((P, NG, NG), np.uint16)
    for g in range(NG):
        ltc[:, g, :g] = 1
    ltc = ltc.reshape(P, NG * NG).astype(np.uint16)
    consts = np.ones((2, N), np.float32)
    consts[1] = -1.0
    in_maps = []
    for c in range(NCORES):
        sidx = [c * SPC + k for k in range(SPC)]
        noiseTn = np.stack([-noise[s].T for s in sidx]).astype(np.float32)
        cloudT = np.stack([cloud[s].T for s in sidx]).astype(np.float32)
        cloudR = np.stack([
            cloud[s].reshape(NG, P, D).transpose(1, 0, 2).reshape(P, NG * D)
            for s in sidx]).astype(np.float32)
        noiseR = np.stack([
            noise[s].reshape(NG, P, D).transpose(1, 0, 2).reshape(P, NG * D)
            for s in sidx]).astype(np.float32)
        tv = np.array([[t[s]] for s in sidx], np.float32)
        m = {
            "noiseTn": np.ascontiguousarray(noiseTn),
            "cloudT": np.ascontiguousarray(cloudT),
            "cloudR": np.ascontiguousarray(cloudR),
            "noiseR": np.ascontiguousarray(noiseR),
            "tv": tv, "ltc": ltc, "consts": consts,
        }
        for k in range(SPC):
            m[f"cloudJ{k}"] = np.ascontiguousarray(
                cloud[sidx[k]].astype(np.float32))
        in_maps.append(m)
    return in_maps


def _host_post(results, B):
    out = np.zeros((2, B, N, D), np.float32)
    for c in range(NCORES):
        o = results[c]["out"]  # [SPC, 2, P, NG*D]
        for k in range(SPC):
            s = c * SPC + k
            for which in range(2):
                arr = o[k, which].reshape(P, NG, D).transpose(1, 0, 2)
                out[which, s] = arr.reshape(N, D)
    return out


def kernel(cloud, noise, t):
    from concourse import bass_utils
    cloud = np.asarray(cloud, np.float32)
    noise = np.asarray(noise, np.float32)
    t = np.asarray(t, np.float32)
    nc = _get_nc()
    in_maps = _host_prep(cloud, noise, t)
    res = bass_utils.run_bass_kernel_spmd(nc, in_maps,
                                          core_ids=list(range(NCORES)))
    return _host_post(res.results, cloud.shape[0])
